# revision 9
# baseline (speedup 1.0000x reference)
"""Fused multi-head attention for Trainium2, 8-core data-parallel. v4.

v7/v5/v4 changes over v3:
  * HW-proven epilogue (cross-partition moves via DMA; engines cannot shift
    partitions on hardware even though CoreSim models it).
  * qk_pair(t+1) emitted mid j-loop so PE has independent work queued at the
    pair boundary.
  * xT loaded as two half-L DMAs so the first projection starts earlier.
  * Output projection split: k=0..4 partials run during pair-5 attention
    (PE idle there), only k=5 + bias + a DVE add remain after the last
    epilogue.

v3 changes over v2:
  * The per-pair den partition-move DMAs are gone: the reciprocal reads the
    denominator row directly at partition 64 (32-aligned start partitions are
    legal engine APs) and writes partition 0 for partition_broadcast.
  * w_qkv columns re-ordered pair-major on the host (Q_t|K_t blocks, then V),
    so the pair-0 QK projection starts after one small DMA + xT.
  * xT loaded as one DMA; V/out staging copies moved to DVE so the scalar
    engine runs the exp stream uncontended.
  * Emission order: qk_pair(0) -> V -> per-pair [attention t | qk_pair(t+1) |
    epilogue t] -> output projection.
"""

import sys

import numpy as np

sys.path.insert(0, "/opt/trn_rl_repo")

B, L, D, H, HD = 8, 1024, 768, 12, 64
E = 3 * D
SCALE = HD ** -0.5
P = 128
KC = D // P          # 6 contraction chunks of 128 over d
LT = L // P          # 8 l/m partition tiles
NP = H // 2          # 6 head pairs
NCORES = 8
NEG = -30000.0       # mask bias; exp(NEG + s) == 0 in fp32

_cached = {}


def _build_program(reps=1):
    import concourse.tile as tile
    from concourse import bacc, mybir

    f32 = mybir.dt.float32
    f32r = mybir.dt.float32r
    bf16 = mybir.dt.bfloat16
    AF = mybir.ActivationFunctionType

    nc = bacc.Bacc(trn_type="TRN2", target_bir_lowering=False, debug=False)

    xT_d = nc.declare_dram_parameter("xT", [P, KC * L], bf16, isOutput=False)
    w1T_d = nc.declare_dram_parameter("w1T", [P, KC * E], bf16, isOutput=False)
    w2T_d = nc.declare_dram_parameter("w2T", [P, KC * D], f32r, isOutput=False)
    b2_d = nc.declare_dram_parameter("b2", [1, D], f32r, isOutput=False)
    mbias_d = nc.declare_dram_parameter("mbias", [P, LT], f32, isOutput=False)
    ones_d = nc.declare_dram_parameter("ones", [1, P], f32r, isOutput=False)
    out_d = nc.declare_dram_parameter("out", [P, LT * D], f32, isOutput=True)

    with tile.TileContext(nc) as tc:
      from contextlib import ExitStack

      # xT/w1T double-buffered by rep parity: rep n+1's input DMAs and first
      # projections overlap rep n's tail instead of waiting on SBUF reuse.
      _outer = ExitStack()
      _pxw = _outer.enter_context(tc.tile_pool(name="xw", bufs=1))
      _xT_bufs = [
          _pxw.tile([P, KC, L], bf16, name=f"xTb{_p}") for _p in range(2)
      ]
      _w1_bufs = [
          _pxw.tile([P, KC, E], bf16, name=f"w1b{_p}") for _p in range(2)
      ]
      for _rep in range(reps):
        with ExitStack() as ctx:
            persist = ctx.enter_context(tc.tile_pool(name="persist", bufs=1))
            # qkvT: separate tiles pair-major: 2t = Q pair t, 2t+1 = K pair t
            qkT_ts = [persist.tile([P, L], bf16, name=f"qkT{i}") for i in range(2 * KC)]
            # V' = [V | ones x 32] per head: the 32 ones columns make the AV
            # matmul emit 32 replicated denominator rows (psum rows 64..95)
            # at zero extra PE cost (matmul cost is N-driven)
            VC = HD + 32
            V_ts = [persist.tile([P, H * VC], bf16, name=f"Vt{i}") for i in range(LT)]
            V_vs = [
                Vt[:].rearrange("p (h c) -> p h c", c=VC) for Vt in V_ts
            ]
            OT_ts = [persist.tile([P, L], f32r, name=f"OTt{i}") for i in range(KC)]
            bias_sb = persist.tile([P, LT], f32)         # mask bias per key pos
            ones_sb = persist.tile([1, P], f32r)

            # f32r memset trips the walrus ISA verifier; load ones from DRAM
            nc.sync.dma_start(out=ones_sb[0:1, :], in_=ones_d.ap())
            for Vv in V_vs:
                nc.gpsimd.memset(Vv[:, :, HD:VC], 1.0)

            xT_sb = _xT_bufs[_rep % 2]
            w1T_sb = _w1_bufs[_rep % 2]
            xT_r = xT_d.ap().rearrange("p (k l) -> p k l", l=L)
            w1T_r = w1T_d.ap().rearrange("p (k e) -> p k e", e=E)
            nc.sync.dma_start(out=xT_sb[:, :, 0:512], in_=xT_r[:, :, 0:512])
            # pair-0 QK columns first, then V columns, then remaining pairs
            nc.sync.dma_start(
                out=w1T_sb[:, :, 0:256], in_=w1T_r[:, :, 0:256]
            )
            nc.sync.dma_start(out=xT_sb[:, :, 512:L], in_=xT_r[:, :, 512:L])
            nc.sync.dma_start(
                out=w1T_sb[:, :, 2 * D : E], in_=w1T_r[:, :, 2 * D : E]
            )
            for t in range(1, NP):
                nc.sync.dma_start(
                    out=w1T_sb[:, :, 256 * t : 256 * (t + 1)],
                    in_=w1T_r[:, :, 256 * t : 256 * (t + 1)],
                )
            nc.sync.dma_start(out=bias_sb[:], in_=mbias_d.ap())

            with tc.tile_pool(name="late", bufs=1) as pL:
                w2Tb_sb = pL.tile([P, KC + 1, D], f32r)

                with tc.tile_pool(name="ps", bufs=2, space="PSUM") as psS, tc.tile_pool(
                    name="psO", bufs=1, space="PSUM"
                ) as psO, tc.tile_pool(name="pt", bufs=2) as ptp, tc.tile_pool(
                    name="norm", bufs=1
                ) as pn, tc.tile_pool(name="outp", bufs=2) as pout:

                    def qk_pair(t):
                        # Q tile (et=2t) and K tile (et=2t+1): [128, L] each
                        for et in (2 * t, 2 * t + 1):
                            ps = psS.tile([P, L], f32, tag="ps")
                            for c in range(2):
                                for k in range(KC):
                                    nc.tensor.matmul(
                                        ps[:, c * 512 : (c + 1) * 512],
                                        lhsT=w1T_sb[:, k, et * P : (et + 1) * P],
                                        rhs=xT_sb[:, k, c * 512 : (c + 1) * 512],
                                        start=(k == 0),
                                        stop=(k == KC - 1),
                                    )
                            nc.vector.tensor_copy(qkT_ts[et][:], ps[:])

                    qk_pair(0)

                    # ---- V projection: V[l, dv] = x @ w1_v.T ----
                    for i in range(LT):
                        ps = psS.tile([P, L], f32, tag="ps")
                        for c0, cw in ((0, 512), (512, 256)):
                            for k in range(KC):
                                nc.tensor.matmul(
                                    ps[:, c0 : c0 + cw],
                                    lhsT=xT_sb[:, k, i * P : (i + 1) * P],
                                    rhs=w1T_sb[:, k, 2 * D + c0 : 2 * D + c0 + cw],
                                    start=(k == 0),
                                    stop=(k == KC - 1),
                                )
                        for c in range(2):
                            nc.vector.tensor_copy(
                                V_vs[i][:, 6 * c : 6 * (c + 1), 0:HD],
                                ps[:, c * 384 : (c + 1) * 384].rearrange(
                                    "p (h q) -> p h q", q=HD
                                ),
                            )

                    # w2 + bias prefetch (off the critical path)
                    nc.sync.dma_start(
                        out=w2Tb_sb[:, 0:KC, :],
                        in_=w2T_d.ap().rearrange("p (k f) -> p k f", f=D),
                    )
                    nc.sync.dma_start(out=w2Tb_sb[0:1, KC, :], in_=b2_d.ap())

                    for t in range(NP):
                        # [128,L] so rows 64:96 exist for the 32-wide den
                        # shuffles (only row 64 holds real data)
                        oA = psO.tile([P, L], f32, tag="oA")
                        oB = psO.tile([P, L], f32, tag="oB")
                        otiles = (oA, oB)
                        jh_order = (
                            [(j, hh) for j in range(LT) for hh in (0, 1)]
                            if t < NP - 1
                            else [(j, hh) for hh in (1, 0) for j in range(LT)]
                        )
                        for j, hh in jh_order:
                            if True:
                                h = 2 * t + hh
                                ro = 64 * hh
                                sps = psS.tile([P, L], f32, tag="ps")
                                for c in range(2):
                                    nc.tensor.matmul(
                                        sps[:, c * 512 : (c + 1) * 512],
                                        lhsT=qkT_ts[2 * t + 1][
                                            ro : ro + 64, j * P : (j + 1) * P
                                        ],
                                        rhs=qkT_ts[2 * t][
                                            ro : ro + 64, c * 512 : (c + 1) * 512
                                        ],
                                        start=True,
                                        stop=True,
                                    )
                                pt_t = ptp.tile([P, L], bf16, tag=f"pt{hh}")
                                nc.scalar.activation(
                                    pt_t[:],
                                    sps[:],
                                    AF.Exp,
                                    bias=bias_sb[:, j : j + 1],
                                    scale=SCALE,
                                )
                                for c in range(2):
                                    nc.tensor.matmul(
                                        otiles[hh][0:96, c * 512 : (c + 1) * 512],
                                        lhsT=V_vs[j][:, h, :],
                                        rhs=pt_t[:, c * 512 : (c + 1) * 512],
                                        start=(j == 0),
                                        stop=(j == LT - 1),
                                    )
                            if j == 2 and hh == 1 and t + 1 < NP:
                                qk_pair(t + 1)
                        # stage O' to SBUF fast (frees the PSUM accumulators);
                        # rows 0..63 are O'.T, row 64 is the softmax denominator
                        # merged epilogue. HW partition rules (probed):
                        # tensor_copy/stream_shuffle may cross 32-aligned
                        # partition bases (and shuffle may read PSUM);
                        # partition_broadcast writes start at partition 0;
                        # reciprocal must not cross partitions.
                        osAB = pn.tile([P, L], f32, tag="osAB")
                        nc.vector.tensor_copy(osAB[0:64, :], oA[0:64, :])
                        nc.vector.tensor_copy(osAB[64:128, :], oB[0:64, :])
                        den0 = pn.tile([32, 2, L], f32, tag="den0")
                        Z32 = [0] * 32
                        nc.vector.stream_shuffle(
                            den0[0:32, 1, :], oB[64:96, :], Z32
                        )
                        nc.vector.stream_shuffle(
                            den0[0:32, 0, :], oA[64:96, :], Z32
                        )
                        denr = pn.tile([1, 2, L], f32, tag="denr")
                        nc.vector.reciprocal_approx_fast(
                            denr[0:1, 1, :], den0[0:1, 1, :]
                        )
                        nc.vector.reciprocal_approx_fast(
                            denr[0:1, 0, :], den0[0:1, 0, :]
                        )
                        # broadcast head-B reciprocal to all 128 partitions,
                        # then overwrite rows 0..63 with head-A's
                        rep2 = pn.tile([P, L], f32, tag="rep2")
                        nc.gpsimd.partition_broadcast(
                            rep2[0:128, :], denr[0:1, 1, :], channels=128
                        )
                        nc.gpsimd.partition_broadcast(
                            rep2[0:64, :], denr[0:1, 0, :], channels=64
                        )
                        nc.vector.tensor_mul(
                            OT_ts[t][:, :], osAB[:, :], rep2[:, :]
                        )


                    # ---- output projection ----
                    out_r = out_d.ap().rearrange("p (i f) -> p i f", f=D)
                    for i in range(LT):
                        ps = psS.tile([P, L], f32, tag="ps")
                        for c0, cw in ((0, 512), (512, 256)):
                            for k in range(KC):
                                nc.tensor.matmul(
                                    ps[:, c0 : c0 + cw],
                                    lhsT=OT_ts[k][:, i * P : (i + 1) * P],
                                    rhs=w2Tb_sb[:, k, c0 : c0 + cw],
                                    start=(k == 0),
                                    stop=False,
                                )
                            # bias via ones-row rank-1 matmul
                            nc.tensor.matmul(
                                ps[:, c0 : c0 + cw],
                                lhsT=ones_sb[0:1, 0:P],
                                rhs=w2Tb_sb[0:1, KC, c0 : c0 + cw],
                                start=False,
                                stop=True,
                            )
                        ob = pout.tile([P, D], f32, tag="ob")
                        nc.vector.tensor_copy(ob[:], ps[:, 0:D])
                        nc.sync.dma_start(out=out_r[:, i, :], in_=ob[:])

      _outer.close()

    nc.compile()
    return nc


def _get_program(reps=1):
    key = f"nc{reps}"
    if key not in _cached:
        _cached[key] = _build_program(reps)
    return _cached[key]


def _prep_inputs(x, attn_mask, w_qkv, w_proj, b_proj):
    import ml_dtypes

    bf = ml_dtypes.bfloat16
    x = np.asarray(x, dtype=np.float32)
    attn_mask = np.asarray(attn_mask)
    w1T = np.ascontiguousarray(np.asarray(w_qkv, np.float32).T)        # [768, 2304]
    # pair-major column order: Q_t | K_t blocks, then all V columns
    blocks = []
    for t in range(NP):
        blocks.append(w1T[:, 128 * t : 128 * (t + 1)])                 # Q pair t
        blocks.append(w1T[:, D + 128 * t : D + 128 * (t + 1)])         # K pair t
    blocks.append(w1T[:, 2 * D :])
    w1T = np.ascontiguousarray(np.concatenate(blocks, axis=1))
    w2T = np.ascontiguousarray(np.asarray(w_proj, np.float32).T)       # [768, 768]
    b2 = np.asarray(b_proj, np.float32)[None, :]

    def swz(a, inner):
        # [KC*P, inner] -> [P, KC*inner], partition-major contiguous
        return np.ascontiguousarray(
            a.reshape(KC, P, inner).transpose(1, 0, 2).reshape(P, KC * inner)
        )

    w1Ts = swz(w1T, E).astype(bf)
    w2Ts = swz(w2T, D)
    b2 = np.ascontiguousarray(b2)
    ones = np.ones((1, P), np.float32)
    in_maps = []
    for b in range(B):
        xT = swz(np.ascontiguousarray(x[b].T), L).astype(bf)           # [128, 6144]
        mb = NEG * (1 - attn_mask[b].astype(np.float32))               # [1024]
        mbs = np.ascontiguousarray(mb.reshape(LT, P).T.astype(np.float32))
        in_maps.append(
            {
                "xT": xT,
                "w1T": w1Ts,
                "w2T": w2Ts,
                "b2": b2,
                "mbias": mbs,
                "ones": ones,
            }
        )
    return in_maps


def run(x, attn_mask, w_qkv, w_proj, b_proj, trace=False, **spmd_kwargs):
    from concourse.bass_utils import run_bass_kernel_spmd

    nc = _get_program()
    in_maps = _prep_inputs(x, attn_mask, w_qkv, w_proj, b_proj)
    res = run_bass_kernel_spmd(
        nc, in_maps, list(range(NCORES)), trace=trace, **spmd_kwargs
    )
    outs = []
    for b in range(B):
        o = np.asarray(res.results[b]["out"]).astype(np.float32)   # [128, 8*768]
        outs.append(
            o.reshape(P, LT, D).transpose(1, 0, 2).reshape(L, D)
        )
    return np.stack(outs, axis=0).astype(np.float32), res


def kernel(x, attn_mask, w_qkv, w_proj, b_proj):
    out, _ = run(x, attn_mask, w_qkv, w_proj, b_proj)
    return out
